# revision 2
# baseline (speedup 1.0000x reference)
"""Trainium2 Bass kernel for the ChaosModulator recurrence, v3.

Device cost model (measured on this axon-tunneled target):
  DVE/ACT op ~ 4.5-6us fixed + 3.5-10ns per free-dim element; DMA ~30-50GB/s.
So: minimize instruction count and HBM bytes, spread passes across engines.

Math (s = 2z-1 + tanh-term form, X4 = 2*x prescaled fp16 on host):
  e_t = X4_t - 0.875*s_{t-1}^2          [DVE, custom CHAOS_E, 1 op/step]
  h_t = tanh(0.25*e_t + 0.875)          [ACT activation, 1 op/step]
  s_t = 0.5*s_{t-1} + h_t               [DVE stt, 1 op/step]
  U_t = s_t + X4_t                      [GPSIMD bulk, 1 op per 4 steps]
  u_t = 0.25*U_t                        [host, folded into fp16->f32 convert]

Blocking: B=16-step blocks, W=4 warmup (contraction 0.5/step; validated
rel err 3.6e-4 incl fp16).  All 256 blocks of a q-row advance in lockstep,
so a whole half (8 q-rows x 256 blocks = 2048 chains/partition) needs only
L=20 sequential steps x 3 engine-parallel ops.  x lives resident in SBUF
[128, 16, 257*16] fp16 (front pad 4, tail pad 12); step k touches padded
index n*16+k = natural slice [:, :, n, k] of the [257,16] row.  U
overwrites X4 in place (positions mod 16 disjoint from later reads).
Two q-halves pipeline DMA against compute.

Sharding: 8 cores x 2048 sequences (batch dim).
"""

import numpy as np

import concourse.bacc as bacc
import concourse.dve_ops as dve_ops
import concourse.mybir as mybir
from concourse.bass_utils import run_bass_kernel_spmd
from concourse.dve_spec import Spec, Src0, Src1, _has_src1, lower, sq
from concourse.dve_uop import DveOpSpec
from concourse.tile import TileContext

F16 = mybir.dt.float16
F32 = mybir.dt.float32

P = 128
Q = 16
NSEQ = P * Q        # 2048 sequences per core
T = 4096
B = 16
W = 4
L = B + W           # 20
NBLK = T // B       # 256
ROW = (B + T) // B * B + B  # 4112 = 257*16 (pad 4 front, 12 tail)
NH = 2
QH = Q // NH        # 8
C = QH * NBLK       # 2048 chain columns per half per partition
NCORES = 8
UCHUNK = 4          # output steps per bulk-u op (= s_hist slots)

_MULT = mybir.AluOpType.mult
_ADD = mybir.AluOpType.add

U_ENGINE = "gpsimd"  # "gpsimd" | "vector"
OUT_SCALE = 0.25     # u = OUT_SCALE * (device output U = s + X4)


def _register_custom_ops():
    """CHAOS_E: out = in1 - s0*in0^2 (idempotent registration)."""
    if "CHAOS_E" in dve_ops._SUB_OPCODE_FOR_NAME:
        by = {op.name: op for op in dve_ops.OPS}
        return by["CHAOS_E"]
    from concourse.dve_spec import C0

    spec_e = Spec(
        body=Src1 - C0 * sq(Src0),
        reference=lambda in0, in1, s0: in1 - s0 * in0 * in0,
    )
    op = dve_ops.DveOp("CHAOS_E", spec_e, subdim=False, uops_sha={})
    dve_ops.OPS.append(op)
    dve_ops.CUSTOM_DVE_SPECS["CHAOS_E"] = spec_e
    dve_ops._SUB_OPCODE_FOR_NAME["CHAOS_E"] = (
        dve_ops._CUSTOM_DVE_ROW_BASE + len(dve_ops.OPS) - 1
    )
    for ver in ("v3", "v4"):
        try:
            s = DveOpSpec(
                name="CHAOS_E",
                opcode=dve_ops.get_dve_sub_opcode("CHAOS_E"),
                uops=lower(spec_e, ver=ver),
                rd1_en=_has_src1(spec_e),
            )
            op.uops_sha[ver] = s.sha(ver)
        except Exception:
            pass
    return op


def _build_nc():
    CHAOS_E = _register_custom_ops()
    nc = bacc.Bacc("TRN2", target_bir_lowering=False, debug=False)

    x = nc.dram_tensor("x", [NSEQ, T], F16, kind="ExternalInput")
    z0 = nc.dram_tensor("z0", [NSEQ], F32, kind="ExternalInput")
    u = nc.dram_tensor("u", [NSEQ, T], F16, kind="ExternalOutput")

    xr = x[:, :].rearrange("(p q) t -> p q t", p=P)   # [128, 16, 4096]
    ur = u[:, :].rearrange("(p q) t -> p q t", p=P)
    z0r = z0[:].rearrange("(p q) -> p q", p=P)        # [128, 16]

    with TileContext(nc) as tc:
        with tc.tile_pool(name="cp", bufs=1) as cp:
            # ---- constants / init ----
            z0_t = cp.tile([P, Q], F32)
            nc.sync.dma_start(out=z0_t[:, :], in_=z0r)
            s_init = cp.tile([P, Q, 1], F16)
            nc.vector.tensor_scalar(
                out=s_init[:, :, 0], in0=z0_t[:, :],
                scalar1=4.0, scalar2=-2.0, op0=_MULT, op1=_ADD,
            )
            bias_t = cp.tile([P, 1], F32)
            nc.vector.memset(bias_t[:, :], 0.875)

            # ---- resident X4 tile, both halves ----
            Xt = cp.tile([P, Q, ROW], F16)
            nc.vector.memset(Xt[:, :, 0:W], 0.0)   # x[t<0] = 0
            for qi in range(Q):
                eng = nc.sync if qi % 2 == 0 else nc.scalar
                eng.dma_start(out=Xt[:, qi, W:W + T], in_=xr[:, qi, :])

            uengine = nc.gpsimd if U_ENGINE == "gpsimd" else nc.vector

            for hf in range(NH):
                qlo = hf * QH
                Xh = Xt[:, qlo:qlo + QH, :]                       # [P,QH,ROW]
                Xn = Xh.rearrange("p q (n j) -> p q n j", j=B)    # [P,QH,257,16]

                def kview(j):
                    """[P, QH, 256] at padded index n*16 + j."""
                    if j < B:
                        return Xn[:, :, 0:NBLK, j]
                    return Xn[:, :, 1:NBLK + 1, j - B]

                def jview(j0):
                    """[P, QH, 256, UCHUNK] at padded n*16 + [j0, j0+4)."""
                    if j0 + UCHUNK <= B:
                        return Xn[:, :, 0:NBLK, j0:j0 + UCHUNK]
                    return Xn[:, :, 1:NBLK + 1, j0 - B:j0 - B + UCHUNK]

                sh = cp.tile([P, UCHUNK, C], F16, name=f"sh{hf}")
                e_t = cp.tile([P, QH, NBLK], F16, name=f"e{hf}")
                h_t = cp.tile([P, QH, NBLK], F16, name=f"h{hf}")
                s3 = sh[:, :, :].rearrange("p s (q n) -> p q n s", q=QH)

                def sl(k):
                    """state slot for step k as [P, QH, NBLK]."""
                    return sh[:, k % UCHUNK, :].rearrange(
                        "p (q n) -> p q n", q=QH)

                nc.vector.memset(sh[:, UCHUNK - 1, :], 0.0)  # s before k=0
                nc.vector.tensor_copy(out=e_t[:, :, :], in_=kview(0))

                for k in range(L):
                    nc.scalar.activation(
                        out=h_t[:, :, :], in_=e_t[:, :, :],
                        func=mybir.ActivationFunctionType.Tanh,
                        bias=bias_t[:, :], scale=0.25,
                    )
                    nc.vector.scalar_tensor_tensor(
                        out=sl(k), in0=sl(k - 1), scalar=0.5,
                        in1=h_t[:, :, :], op0=_MULT, op1=_ADD,
                    )
                    if k == W - 1:
                        # block 0 of each sequence takes the true z0 state
                        nc.vector.tensor_copy(
                            out=sh[:, k % UCHUNK, :].rearrange(
                                "p (q n) -> p q n", q=QH)[:, :, 0:1],
                            in_=s_init[:, qlo:qlo + QH, :],
                        )
                    if k < L - 1:
                        nc.vector._custom_dve(
                            CHAOS_E, out=e_t[:, :, :], in0=sl(k),
                            in1=kview(k + 1), s0=0.875,
                        )
                    if k >= W + UCHUNK - 1 and (k - W - UCHUNK + 1) % UCHUNK == 0:
                        # U(j0..j0+4) = s_hist + X4, in place (k = 7,11,15,19)
                        j0 = k - UCHUNK + 1
                        uengine.tensor_tensor(
                            out=jview(j0), in0=s3, in1=jview(j0), op=_ADD,
                        )

                for qi in range(QH):
                    eng = nc.sync if qi % 2 == 0 else nc.scalar
                    eng.dma_start(
                        out=ur[:, qlo + qi, :], in_=Xt[:, qlo + qi, W:W + T]
                    )

    nc.compile()
    return nc


_NC = None


def _get_nc():
    global _NC
    if _NC is None:
        _NC = _build_nc()
    return _NC


def _shard_inputs(x, z0):
    """Host prep: X4 = 2*x fp16 shards (u = 0.25*(s + X4) on readback)."""
    in_maps = []
    for i in range(NCORES):
        xs = (2.0 * x[4 * i:4 * (i + 1)].reshape(NSEQ, T)).astype(np.float16)
        zs = np.ascontiguousarray(
            z0[4 * i:4 * (i + 1)].reshape(NSEQ), np.float32)
        in_maps.append({"x": np.ascontiguousarray(xs), "z0": zs})
    return in_maps


def kernel(x: np.ndarray, z0: np.ndarray) -> np.ndarray:
    x = np.ascontiguousarray(x, dtype=np.float32)      # (32, 512, 4096)
    z0 = np.ascontiguousarray(z0, dtype=np.float32)    # (32, 512)
    nc = _get_nc()
    in_maps = _shard_inputs(x, z0)
    res = run_bass_kernel_spmd(nc, in_maps, core_ids=list(range(NCORES)))
    out = np.empty((32, 512, T), np.float32)
    for i in range(NCORES):
        out[4 * i:4 * (i + 1)] = (
            res.results[i]["u"].astype(np.float32) * 0.25
        ).reshape(4, 512, T)
    return out


# revision 7
# speedup vs baseline: 1.0259x; 1.0259x over previous
"""Trainium2 Bass kernel for the ChaosModulator recurrence, v3.

Device cost model (measured on this axon-tunneled target):
  DVE/ACT op ~ 4.5-6us fixed + 3.5-10ns per free-dim element; DMA ~30-50GB/s.
So: minimize instruction count and HBM bytes, spread passes across engines.

Math (s = 2z-1 + tanh-term form, X4 = 2*x prescaled fp16 on host):
  e_t = X4_t - 0.875*s_{t-1}^2          [DVE, custom CHAOS_E, 1 op/step]
  h_t = tanh(0.25*e_t + 0.875)          [ACT activation, 1 op/step]
  s_t = 0.5*s_{t-1} + h_t               [DVE stt, 1 op/step]
  U_t = s_t + X4_t                      [GPSIMD bulk, 1 op per 4 steps]
  u_t = 0.25*U_t                        [host, folded into fp16->f32 convert]

Blocking: B=16-step blocks, W=4 warmup (contraction 0.5/step; validated
rel err 3.6e-4 incl fp16).  All 256 blocks of a q-row advance in lockstep,
so a whole half (8 q-rows x 256 blocks = 2048 chains/partition) needs only
L=20 sequential steps x 3 engine-parallel ops.  x lives resident in SBUF
[128, 16, 257*16] fp16 (front pad 4, tail pad 12); step k touches padded
index n*16+k = natural slice [:, :, n, k] of the [257,16] row.  U
overwrites X4 in place (positions mod 16 disjoint from later reads).
Two q-halves pipeline DMA against compute.

Sharding: 8 cores x 2048 sequences (batch dim).
"""

import numpy as np

import concourse.bacc as bacc
import concourse.dve_ops as dve_ops
import concourse.mybir as mybir
from concourse.bass_utils import run_bass_kernel_spmd
from concourse.dve_spec import Spec, Src0, Src1, _has_src1, lower, sq
from concourse.dve_uop import DveOpSpec
from concourse.tile import TileContext

F16 = mybir.dt.float16
F32 = mybir.dt.float32

P = 128
Q = 16
NSEQ = P * Q        # 2048 sequences per core
T = 4096
B = 16
W = 4
L = B + W           # 20
NBLK = T // B       # 256
ROW = (B + T) // B * B + B  # 4112 = 257*16 (pad 4 front, 12 tail)
NH = 2
QH = Q // NH        # 8
C = QH * NBLK       # 2048 chain columns per half per partition
NCORES = 8
UCHUNK = 4          # output steps per bulk-u op (= s_hist slots)

_MULT = mybir.AluOpType.mult
_ADD = mybir.AluOpType.add

U_ENGINE = "gpsimd"  # "gpsimd" | "vector"
OUT_SCALE = 0.25     # u = OUT_SCALE * (device output U = s + X4)


def _register_custom_ops():
    """CHAOS_E: out = in1 - s0*in0^2 (idempotent registration)."""
    if "CHAOS_E" in dve_ops._SUB_OPCODE_FOR_NAME:
        by = {op.name: op for op in dve_ops.OPS}
        return by["CHAOS_E"]
    from concourse.dve_spec import C0

    spec_e = Spec(
        body=Src1 - C0 * sq(Src0),
        reference=lambda in0, in1, s0: in1 - s0 * in0 * in0,
    )
    op = dve_ops.DveOp("CHAOS_E", spec_e, subdim=False, uops_sha={})
    dve_ops.OPS.append(op)
    dve_ops.CUSTOM_DVE_SPECS["CHAOS_E"] = spec_e
    dve_ops._SUB_OPCODE_FOR_NAME["CHAOS_E"] = (
        dve_ops._CUSTOM_DVE_ROW_BASE + len(dve_ops.OPS) - 1
    )
    for ver in ("v3", "v4"):
        try:
            s = DveOpSpec(
                name="CHAOS_E",
                opcode=dve_ops.get_dve_sub_opcode("CHAOS_E"),
                uops=lower(spec_e, ver=ver),
                rd1_en=_has_src1(spec_e),
            )
            op.uops_sha[ver] = s.sha(ver)
        except Exception:
            pass
    return op


def _build_nc():
    CHAOS_E = _register_custom_ops()
    nc = bacc.Bacc("TRN2", target_bir_lowering=False, debug=False)

    x = nc.dram_tensor("x", [NSEQ, T], F16, kind="ExternalInput")
    z0 = nc.dram_tensor("z0", [NSEQ], F32, kind="ExternalInput")
    u = nc.dram_tensor("u", [NSEQ, T], F16, kind="ExternalOutput")

    xr = x[:, :].rearrange("(p q) t -> p q t", p=P)   # [128, 16, 4096]
    ur = u[:, :].rearrange("(p q) t -> p q t", p=P)
    z0r = z0[:].rearrange("(p q) -> p q", p=P)        # [128, 16]

    with TileContext(nc) as tc:
        with tc.tile_pool(name="cp", bufs=1) as cp:
            # ---- constants / init ----
            z0_t = cp.tile([P, Q], F32)
            nc.sync.dma_start(out=z0_t[:, :], in_=z0r)
            s_init = cp.tile([P, Q, 1], F16)
            nc.vector.tensor_scalar(
                out=s_init[:, :, 0], in0=z0_t[:, :],
                scalar1=4.0, scalar2=-2.0, op0=_MULT, op1=_ADD,
            )
            bias_t = cp.tile([P, 1], F32)
            nc.vector.memset(bias_t[:, :], 0.875)

            # ---- resident X4 tile, both halves ----
            Xt = cp.tile([P, Q, ROW], F16)
            nc.vector.memset(Xt[:, :, 0:W], 0.0)   # x[t<0] = 0
            # 0.5 MiB chunks, alternating the two HWDGE rings, for deeper
            # DMA-queue concurrency (measured ~2x bw vs few large DMAs)
            HT = T // 2
            for qi in range(Q):
                for ci in range(2):
                    eng = nc.sync if (2 * qi + ci) % 2 == 0 else nc.scalar
                    eng.dma_start(
                        out=Xt[:, qi, W + ci * HT:W + (ci + 1) * HT],
                        in_=xr[:, qi, ci * HT:(ci + 1) * HT],
                    )

            uengine = nc.gpsimd if U_ENGINE == "gpsimd" else nc.vector

            for hf in range(NH):
                qlo = hf * QH
                Xh = Xt[:, qlo:qlo + QH, :]                       # [P,QH,ROW]
                Xn = Xh.rearrange("p q (n j) -> p q n j", j=B)    # [P,QH,257,16]

                def kview(j):
                    """[P, QH, 256] at padded index n*16 + j."""
                    if j < B:
                        return Xn[:, :, 0:NBLK, j]
                    return Xn[:, :, 1:NBLK + 1, j - B]

                def jview(j0):
                    """[P, QH, 256, UCHUNK] at padded n*16 + [j0, j0+4)."""
                    if j0 + UCHUNK <= B:
                        return Xn[:, :, 0:NBLK, j0:j0 + UCHUNK]
                    return Xn[:, :, 1:NBLK + 1, j0 - B:j0 - B + UCHUNK]

                sh = cp.tile([P, UCHUNK, C], F16, name=f"sh{hf}")
                # double-buffered e/h: breaks the ACT<->DVE WAR serialization
                # (CHAOS_E(k) writes e for k+1 while ACT(k) still reads e(k))
                e_t = [cp.tile([P, QH, NBLK], F16, name=f"e{hf}_{i}")
                       for i in range(2)]
                h_t = [cp.tile([P, QH, NBLK], F16, name=f"h{hf}_{i}")
                       for i in range(2)]
                s3 = sh[:, :, :].rearrange("p s (q n) -> p q n s", q=QH)

                def sl(k):
                    """state slot for step k as [P, QH, NBLK]."""
                    return sh[:, k % UCHUNK, :].rearrange(
                        "p (q n) -> p q n", q=QH)

                nc.vector.memset(sh[:, UCHUNK - 1, :], 0.0)  # s before k=0
                nc.vector.tensor_copy(out=e_t[0][:, :, :], in_=kview(0))

                for k in range(L):
                    nc.scalar.activation(
                        out=h_t[k % 2][:, :, :], in_=e_t[k % 2][:, :, :],
                        func=mybir.ActivationFunctionType.Tanh,
                        bias=bias_t[:, :], scale=0.25,
                    )
                    nc.vector.scalar_tensor_tensor(
                        out=sl(k), in0=sl(k - 1), scalar=0.5,
                        in1=h_t[k % 2][:, :, :], op0=_MULT, op1=_ADD,
                    )
                    if k == W - 1:
                        # block 0 of each sequence takes the true z0 state
                        nc.vector.tensor_copy(
                            out=sh[:, k % UCHUNK, :].rearrange(
                                "p (q n) -> p q n", q=QH)[:, :, 0:1],
                            in_=s_init[:, qlo:qlo + QH, :],
                        )
                    if k < L - 1:
                        nc.vector._custom_dve(
                            CHAOS_E, out=e_t[(k + 1) % 2][:, :, :], in0=sl(k),
                            in1=kview(k + 1), s0=0.875,
                        )
                    if k >= W + UCHUNK - 1 and (k - W - UCHUNK + 1) % UCHUNK == 0:
                        # U(j0..j0+4) = s_hist + X4, in place (k = 7,11,15,19)
                        j0 = k - UCHUNK + 1
                        uengine.tensor_tensor(
                            out=jview(j0), in0=s3, in1=jview(j0), op=_ADD,
                        )

                for qi in range(QH):
                    for ci in range(2):
                        eng = nc.sync if (2 * qi + ci) % 2 == 0 else nc.scalar
                        eng.dma_start(
                            out=ur[:, qlo + qi, ci * HT:(ci + 1) * HT],
                            in_=Xt[:, qlo + qi,
                                   W + ci * HT:W + (ci + 1) * HT],
                        )

    nc.compile()
    return nc


_NC = None


def _get_nc():
    global _NC
    if _NC is None:
        _NC = _build_nc()
    return _NC


def _shard_inputs(x, z0):
    """Host prep: X4 = 2*x fp16 shards (u = 0.25*(s + X4) on readback)."""
    in_maps = []
    for i in range(NCORES):
        xs = (2.0 * x[4 * i:4 * (i + 1)].reshape(NSEQ, T)).astype(np.float16)
        zs = np.ascontiguousarray(
            z0[4 * i:4 * (i + 1)].reshape(NSEQ), np.float32)
        in_maps.append({"x": np.ascontiguousarray(xs), "z0": zs})
    return in_maps


def kernel(x: np.ndarray, z0: np.ndarray) -> np.ndarray:
    x = np.ascontiguousarray(x, dtype=np.float32)      # (32, 512, 4096)
    z0 = np.ascontiguousarray(z0, dtype=np.float32)    # (32, 512)
    nc = _get_nc()
    in_maps = _shard_inputs(x, z0)
    res = run_bass_kernel_spmd(nc, in_maps, core_ids=list(range(NCORES)))
    out = np.empty((32, 512, T), np.float32)
    for i in range(NCORES):
        out[4 * i:4 * (i + 1)] = (
            res.results[i]["u"].astype(np.float32) * 0.25
        ).reshape(4, 512, T)
    return out
